# revision 1
# baseline (speedup 1.0000x reference)
"""Multi-head attention (B=4, S=2048, D=1024, H=16, Hd=64) on 8 NeuronCores.

Sharding: tensor-parallel over heads. Core c owns heads {2c, 2c+1}, i.e. a
128-column slice of Wq/Wk/Wv and the matching 128-row slice of Wo. Each core
computes a full-shape partial output (its heads' contribution through the out
projection); the host sums the 8 partials (f32) plus the exact bias identities
(softmax rows sum to 1 -> bv@Wo + bo added on host; bk cancels in softmax but
is still applied on-device for free).

v2 structure (vs v1): the whole kernel is a single software pipeline over
batches, engineered to keep the PE array continuously busy so the HAM clock
gate stays at 2.4 GHz (v1 ran most matmuls at the cold 1.2 GHz rate):

  * per-batch projection -> attention -> out-projection, with projection and
    out-projection matmuls of neighbouring batches interleaved as "filler"
    units inside the attention chunk loop (PE never idles while ACT does exp).
  * scores for the two heads are issued as a row-tiled pair (h0 rows 0:64,
    h1 rows 64:128 via tile_position auto-derivation) into separate PSUM banks
    of one [128, 2, 512] tile -> they execute concurrently on the PE, and one
    ACT exp (N=1024) covers both heads.
  * softmax normalization: DVE reciprocal_approx_fast (0.7us vs 3.3us for the
    iterative reciprocal), then a col-tiled concurrent pair of K=1 ones-matmul
    broadcasts, one PSUM->SBUF copy, two DVE multiplies.
  * out-projection eviction entirely on DVE (v1 put half on ACT, the exp
    engine), output written bf16 (halves write traffic; host sums in f32).

Device algorithm per core (all matmuls bf16, f32 PSUM):
  1. QT/KT = Wc^T x^T + b  -> SBUF [128=d', 8192=s] bf16 (h0 rows 0:64,
     h1 rows 64:128); V -> SBUF [token, d'] chunks with ones columns for the
     softmax row-sum rows (VE layout [128, chunk, 2, 132]).
  2. Per (batch, q-slice of 512): 16 k-chunks of 128: scores^T pair ->
     exp -> P^T; O^T accumulated per head via [V_h | ones] lhsT (row-sum row
     rides along). Normalize with reciprocal + PE broadcast -> OT bf16.
  3. out_partial = OT^T @ Wo per s-tile -> DRAM bf16.
"""

import os
from contextlib import ExitStack

import numpy as np
import ml_dtypes

import concourse.bass as bass
import concourse.mybir as mybir
import concourse.tile as tile
from concourse import bacc, library_config
from concourse.bass_utils import run_bass_kernel_spmd

B, S, D, H, HD = 4, 2048, 1024, 16, 64
BS = B * S                     # 8192 flattened tokens
NCORES = 8
HPC = H // NCORES              # 2 heads per core
DC = HPC * HD                  # 128-wide weight slice per core

F32 = mybir.dt.float32
BF16 = mybir.dt.bfloat16
EXP = mybir.ActivationFunctionType.Exp
LOG = mybir.ActivationFunctionType.Ln

_BUILT = None
LAST_EXEC_NS = None
LAST_RESULTS = None


def _patch_act_tables():
    """Steer the ACT table-load inserter to the combined exp+ln set.

    The inserter greedily picks the first act-func set containing each
    activation's function, so Exp lands in `exp_and_others` and Ln in
    `natural_log` and the kernel thrashes 2.7us table loads every q-slice.
    Presenting Exp/Ln as available only in `natural_log_exp_and_others`
    (which genuinely contains both) yields a single load at kernel start.
    """
    import concourse.bacc as bacc_mod

    orig = bacc_mod.get_activation_tables
    if getattr(orig, "_combined_exp_ln", False):
        return
    def patched(arch):
        tables = orig(arch)
        for name, fns in tables.items():
            if name != "natural_log_exp_and_others":
                fns.discard(mybir.ActivationFunctionType.Exp)
                fns.discard(mybir.ActivationFunctionType.Ln)
        return tables
    patched._combined_exp_ln = True
    bacc_mod.get_activation_tables = patched


def _build_program():
    _patch_act_tables()
    nc = bacc.Bacc("TRN2", target_bir_lowering=False, debug=False,
                   num_devices=NCORES)

    qT_d = nc.dram_tensor("qT", [D, BS], BF16, kind="ExternalInput").ap()
    kT_d = nc.dram_tensor("kT", [D, BS], BF16, kind="ExternalInput").ap()
    vT_d = nc.dram_tensor("vT", [D, BS], BF16, kind="ExternalInput").ap()
    wq_d = nc.dram_tensor("wq", [D, DC], BF16, kind="ExternalInput").ap()
    wk_d = nc.dram_tensor("wk", [D, DC], BF16, kind="ExternalInput").ap()
    wv_d = nc.dram_tensor("wv", [D, DC], BF16, kind="ExternalInput").ap()
    wo_d = nc.dram_tensor("wo", [DC, D], BF16, kind="ExternalInput").ap()
    bq_d = nc.dram_tensor("bq", [DC, 1], F32, kind="ExternalInput").ap()
    bk_d = nc.dram_tensor("bk", [DC, 1], F32, kind="ExternalInput").ap()
    out_d = nc.dram_tensor("out", [BS, D], BF16, kind="ExternalOutput").ap()

    with tile.TileContext(nc) as tc, ExitStack() as ctx:
        const = ctx.enter_context(tc.tile_pool(name="const", bufs=1))
        persist = ctx.enter_context(tc.tile_pool(name="persist", bufs=1))
        stage = ctx.enter_context(tc.tile_pool(name="stage", bufs=3))
        ptpool = ctx.enter_context(tc.tile_pool(name="ptpool", bufs=6))
        npool = ctx.enter_context(tc.tile_pool(name="npool", bufs=2))
        ostage = ctx.enter_context(tc.tile_pool(name="ostage", bufs=3))
        # PSUM: psc 2 slots x 2 banks (scores double-buffer)
        #       acc 2 slots x 1 bank (proj accum / out-proj / bcast)
        #       pop 2 slots x 1 bank (per-head O^T accumulators)
        pscp = ctx.enter_context(tc.tile_pool(name="pscp", bufs=2, space="PSUM"))
        accp = ctx.enter_context(tc.tile_pool(name="accp", bufs=2, space="PSUM"))
        pop = ctx.enter_context(tc.tile_pool(name="pop", bufs=2, space="PSUM"))

        # ---- persistent SBUF state -------------------------------------
        QT = persist.tile([128, BS], BF16)          # [d', s]
        KT = persist.tile([128, BS], BF16)
        OT = persist.tile([128, BS], BF16)
        # V extended, per 128-token chunk (free layout [2, 132], abs width 264):
        #   abs cols 0:64    = V_h0          (h0 lhsT = abs 0:65, rsum row 64)
        #   abs col  64      = ones
        #   abs col  68      = ones          (h1 lhsT = abs 68:196, rsum row 0)
        #   abs cols 132:196 = V_h1          (-> h1 lhsT rows 64:128)
        VE = persist.tile([128, 64, 2, 132], BF16)

        # ---- constants --------------------------------------------------
        wq_sb = const.tile([128, 8, DC], BF16)
        wk_sb = const.tile([128, 8, DC], BF16)
        wv_sb = const.tile([128, 8, DC], BF16)
        wo_sb = const.tile([128, D], BF16)
        bq_sb = const.tile([128, 1], F32)
        bk_sb = const.tile([128, 1], F32)
        ones_sb = const.tile([128, 64], BF16)
        warm_sb = const.tile([128, 8], F32)
        nc.vector.memset(ones_sb[:], 1.0)
        nc.vector.memset(warm_sb[:], 0.0)
        nc.sync.dma_start(wq_sb[:], wq_d.rearrange("(c p) d -> p c d", p=128))
        nc.sync.dma_start(wk_sb[:], wk_d.rearrange("(c p) d -> p c d", p=128))
        nc.sync.dma_start(wv_sb[:], wv_d.rearrange("(c p) d -> p c d", p=128))
        nc.sync.dma_start(wo_sb[:], wo_d)
        nc.sync.dma_start(bq_sb[:], bq_d)
        nc.sync.dma_start(bk_sb[:], bk_d)
        nc.vector.memset(VE[:], 0.0)
        nc.vector.memset(VE[:, :, 0, 64:65], 1.0)
        nc.vector.memset(VE[:, :, 0, 68:69], 1.0)
        # preload the exp+log table set during the DMA-bound lead-in
        nc.scalar.activation(warm_sb[:], warm_sb[:], EXP, scale=1.0)
        nc.scalar.activation(warm_sb[:], warm_sb[:], LOG, scale=1.0)

        # ---- work-unit emitters -----------------------------------------
        def emit_qk_unit(which, ss):
            """Project one 512-token slice of Q or K (8 accum matmuls)."""
            srcT, w_sb, b_sb, dstT = (
                (qT_d, wq_sb, bq_sb, QT) if which == "q"
                else (kT_d, wk_sb, bk_sb, KT))
            xt = stage.tile([128, 8, 512], BF16, tag="xT")
            nc.sync.dma_start(
                xt[:],
                srcT.rearrange("(c p) s -> p c s", p=128)[
                    :, :, ss * 512:(ss + 1) * 512],
            )
            ps = accp.tile([128, 512], F32, tag="acc", name="psqk")
            for c in range(8):
                nc.tensor.matmul(ps[:], lhsT=w_sb[:, c], rhs=xt[:, c],
                                 start=(c == 0), stop=(c == 7))
            nc.vector.tensor_scalar_add(
                dstT[:, ss * 512:(ss + 1) * 512], ps[:], b_sb[:])

        def emit_v_load(ss):
            """DMA one 512-token slice of v; returns the staged tile."""
            vt = stage.tile([128, 8, 512], BF16, tag="xT")
            nc.sync.dma_start(
                vt[:],
                vT_d.rearrange("(c p) s -> p c s", p=128)[
                    :, :, ss * 512:(ss + 1) * 512],
            )
            return vt

        def emit_v_unit(vt, ss, st):
            """Project one 128-token chunk of V into the VE layout."""
            chunk = ss * 4 + st
            ps = accp.tile([128, 512], F32, tag="acc", name="psv")
            for c in range(8):
                nc.tensor.matmul(
                    ps[:, 0:DC],
                    lhsT=vt[:, c, st * 128:(st + 1) * 128],
                    rhs=wv_sb[:, c],
                    start=(c == 0), stop=(c == 7))
            nc.vector.tensor_copy(
                VE[:, chunk, :, 0:64],
                ps[:, 0:DC].rearrange("p (a x) -> p a x", a=2))

        def emit_outproj_unit(st_abs):
            """Out-projection for one 128-token tile (2 matmuls, bf16 out)."""
            s0 = st_abs * 128
            osb = ostage.tile([128, D], BF16, tag="osb")
            for ns in range(2):
                ps3 = accp.tile([128, 512], F32, tag="acc", name="ps3")
                nc.tensor.matmul(
                    ps3[:],
                    lhsT=OT[:, s0:s0 + 128],
                    rhs=wo_sb[:, ns * 512:(ns + 1) * 512],
                    start=True, stop=True)
                nc.vector.tensor_copy(osb[:, ns * 512:(ns + 1) * 512], ps3[:])
            nc.sync.dma_start(out_d[s0:s0 + 128, :], osb[:])

        def emit_batch_attention(b, fillers, refill):
            """Attention for one batch as a single software pipeline over all
            64 (q-slice, k-chunk) steps. Per step gi: scores+exp(gi), PV at
            depth 2, and the normalization for a finished q-slice split into
            three stages so the in-order PE queue never waits on the ACT
            reciprocal chain:
              - po evacuation (DVE copies, frees the po PSUM slots),
              - Ln/Exp reciprocal (ACT only),
              - broadcast + multiplies two steps later (rhs ready by then).
            One filler unit runs per step; `refill(qs)` is called when
            q-slice qs's normalization has been fully emitted (its out-proj
            can then be queued as filler)."""
            po_by_qs = {}
            posb_by_qs = {}
            rrb_by_qs = {}
            pts = {}

            def emit_scores_exp(qs, chunk):
                q0 = b * S + qs * 512
                k0 = b * S + chunk * 128
                psc = pscp.tile([128, 2, 512], F32, tag="sc", name="psc")
                # row-tiled concurrent pair: h0 rows 0:64, h1 rows 64:128
                nc.tensor.matmul(
                    psc[:, 0], lhsT=KT[0:64, k0:k0 + 128],
                    rhs=QT[0:64, q0:q0 + 512], start=True, stop=True)
                nc.tensor.matmul(
                    psc[:, 1], lhsT=KT[64:128, k0:k0 + 128],
                    rhs=QT[64:128, q0:q0 + 512], start=True, stop=True)
                pt = ptpool.tile([128, 2, 512], BF16, tag="pt")
                nc.scalar.activation(
                    pt.rearrange("p a x -> p (a x)"),
                    psc.rearrange("p a x -> p (a x)"),
                    EXP, scale=0.125)
                pts[(qs, chunk)] = pt

            def emit_pv(qs, chunk):
                if chunk == 0:
                    po_by_qs[qs] = [
                        pop.tile([128, 512], F32, tag="po", name=f"po{h}")
                        for h in range(HPC)]
                po = po_by_qs[qs]
                pt = pts.pop((qs, chunk))
                ve_flat = VE[:, b * 16 + chunk, :, :].rearrange(
                    "p a x -> p (a x)")
                first = chunk == 0
                last = chunk == 15
                # h0: rows 0:64 = O^T_h0, row 64 = rowsum_h0
                nc.tensor.matmul(
                    po[0][0:65, :], lhsT=ve_flat[:, 0:65], rhs=pt[:, 0],
                    start=first, stop=last)
                # h1: row 0 = rowsum_h1 (ones at abs 68), rows 64:128 = O^T_h1
                nc.tensor.matmul(
                    po[1][:, :], lhsT=ve_flat[:, 68:196], rhs=pt[:, 1],
                    start=first, stop=last)

            def emit_po_evac_and_recip(qs):
                # evacuate the finished accumulators to SBUF (frees both po
                # PSUM slots for the next q-slice's PV) and start the ACT
                # reciprocal 1/rowsum = exp(-ln rowsum); Ln and Exp share the
                # natural_log_exp_and_others table set -> no table switch
                po = po_by_qs.pop(qs)
                posb = [npool.tile([128, 512], F32, tag=f"posb{h}",
                                   name=f"posb{h}")
                        for h in range(HPC)]
                nc.vector.tensor_copy(posb[0][0:65, :], po[0][0:65, :])
                nc.vector.tensor_copy(posb[1][:, :], po[1][:, :])
                posb_by_qs[qs] = posb
                rl = npool.tile([128, 512], F32, tag="rl")
                nc.scalar.activation(rl[64:65, :], posb[0][64:65, :], LOG)
                nc.scalar.activation(rl[0:1, :], posb[1][0:1, :], LOG)
                rrb = npool.tile([128, 512], BF16, tag="rrb")
                nc.scalar.activation(
                    rrb[64:65, :], rl[64:65, :], EXP, scale=-1.0)
                nc.scalar.activation(
                    rrb[0:1, :], rl[0:1, :], EXP, scale=-1.0)
                rrb_by_qs[qs] = rrb

            def emit_norm_pe(qs):
                # col-tiled concurrent ones-broadcast of the reciprocals,
                # then two DVE multiplies write the normalized O^T
                q0 = b * S + qs * 512
                posb = posb_by_qs.pop(qs)
                rrb = rrb_by_qs.pop(qs)
                bcp = accp.tile([128, 512], F32, tag="acc", name="bcp")
                nc.tensor.matmul(bcp[0:64, :], lhsT=ones_sb[64:65, :],
                                 rhs=rrb[64:65, :], start=True, stop=True)
                nc.tensor.matmul(bcp[64:128, :], lhsT=ones_sb[0:1, :],
                                 rhs=rrb[0:1, :], start=True, stop=True)
                nc.vector.tensor_mul(
                    OT[0:64, q0:q0 + 512], posb[0][0:64, :], bcp[0:64, :])
                nc.vector.tensor_mul(
                    OT[64:128, q0:q0 + 512], posb[1][64:128, :],
                    bcp[64:128, :])

            for gi in range(64 + 6):
                if gi < 64:
                    emit_scores_exp(gi // 16, gi % 16)
                d = gi - 2
                if 0 <= d < 64:
                    dq, dc = divmod(d, 16)
                    emit_pv(dq, dc)
                    if dc == 15:
                        emit_po_evac_and_recip(dq)
                e = gi - 5
                if 0 <= e < 64 and e % 16 == 15:
                    emit_norm_pe(e // 16)
                    refill(e // 16)
                run_unit(fillers)

        def run_unit(fillers):
            """Pop and emit one filler unit; a unit may return a list of
            follow-up units which are queued to run next (in order)."""
            if not fillers:
                return
            u = fillers.pop(0)
            r = u()
            if isinstance(r, list):
                fillers[0:0] = r

        def proj_units_for_batch(b):
            # interleave [Q, K, V] per token-slice so the first attention
            # chunks of the batch unblock after ~1/4 of the batch's DMA, and
            # V work spreads evenly across the previous batch's q-slices
            units = []
            for ss_local in range(4):
                ss = b * 4 + ss_local
                units.append(lambda ss=ss: emit_qk_unit("k", ss))
                units.append(lambda ss=ss: emit_qk_unit("q", ss))

                def v_group(ss=ss):
                    vt = emit_v_load(ss)
                    return [lambda st=st, vt=vt, ss=ss: emit_v_unit(vt, ss, st)
                            for st in range(4)]
                units.append(v_group)
            return units

        # ---- main pipeline ----------------------------------------------
        # lead-in: project batch 0 eagerly (DMA-bound)
        lead = proj_units_for_batch(0)
        while lead:
            run_unit(lead)

        for b in range(B):
            fillers = []
            pending_proj = proj_units_for_batch(b + 1) if b + 1 < B else []

            def refill(qs, b=b, fillers=fillers, pending_proj=pending_proj):
                # out-projection of the just-normalized q-slice, then a
                # share of the next batch's projection units
                base = (b * S + qs * 512) // 128
                for k in range(4):
                    fillers.append(lambda st=base + k: emit_outproj_unit(st))
                for _ in range(min(6, len(pending_proj))):
                    fillers.append(pending_proj.pop(0))

            for _ in range(min(6, len(pending_proj))):
                fillers.append(pending_proj.pop(0))

            emit_batch_attention(b, fillers, refill)

            # drain any unfinished fillers / projections at batch end
            while fillers:
                run_unit(fillers)
            while pending_proj:
                run_unit(pending_proj)

    nc.compile()
    return nc


def _get_program():
    global _BUILT
    if _BUILT is None:
        _BUILT = _build_program()
    return _BUILT


def kernel(q, k, v, Wq, bq, Wk, bk, Wv, bv, Wo, bo, trace=None):
    global LAST_EXEC_NS, LAST_RESULTS
    if trace is None:
        trace = os.environ.get("KERNEL_TRACE", "0") == "1"
    bf16 = ml_dtypes.bfloat16

    q2 = np.asarray(q, np.float32).reshape(BS, D)
    k2 = np.asarray(k, np.float32).reshape(BS, D)
    v2 = np.asarray(v, np.float32).reshape(BS, D)
    qT = np.ascontiguousarray(q2.T).astype(bf16)
    kT = np.ascontiguousarray(k2.T).astype(bf16)
    vT = np.ascontiguousarray(v2.T).astype(bf16)

    Wq = np.asarray(Wq, np.float32)
    Wk = np.asarray(Wk, np.float32)
    Wv = np.asarray(Wv, np.float32)
    Wo = np.asarray(Wo, np.float32)
    bq = np.asarray(bq, np.float32)
    bk = np.asarray(bk, np.float32)
    bv = np.asarray(bv, np.float32)
    bo = np.asarray(bo, np.float32)

    in_maps = []
    for c in range(NCORES):
        sl = slice(c * DC, (c + 1) * DC)
        in_maps.append({
            "qT": qT, "kT": kT, "vT": vT,
            "wq": np.ascontiguousarray(Wq[:, sl]).astype(bf16),
            "wk": np.ascontiguousarray(Wk[:, sl]).astype(bf16),
            "wv": np.ascontiguousarray(Wv[:, sl]).astype(bf16),
            "wo": np.ascontiguousarray(Wo[sl, :]).astype(bf16),
            "bq": np.ascontiguousarray(bq[sl]).reshape(DC, 1),
            "bk": np.ascontiguousarray(bk[sl]).reshape(DC, 1),
        })

    nc = _get_program()
    res = run_bass_kernel_spmd(nc, in_maps, list(range(NCORES)), trace=trace)
    LAST_EXEC_NS = res.exec_time_ns
    LAST_RESULTS = res

    out = np.zeros((BS, D), np.float32)
    for c in range(NCORES):
        out += np.asarray(res.results[c]["out"], np.float32)
    out += bv.astype(np.float32) @ Wo + bo          # exact bias identities
    return out.reshape(B, S, D)



# revision 10
# speedup vs baseline: 1.0458x; 1.0458x over previous
"""Multi-head attention (B=4, S=2048, D=1024, H=16, Hd=64) on 8 NeuronCores.

Sharding: tensor-parallel over heads. Core c owns heads {2c, 2c+1}, i.e. a
128-column slice of Wq/Wk/Wv and the matching 128-row slice of Wo. Each core
computes a full-shape partial output (its heads' contribution through the out
projection); the host sums the 8 partials (f32) plus the exact bias identities
(softmax rows sum to 1 -> bv@Wo + bo added on host; bk cancels in softmax but
is still applied on-device for free).

v3 structure (vs v2): one CONTINUOUS software pipeline over all 256
(batch, q-slice, k-chunk) steps -- no per-batch drain barriers. The two
pacing engines (PE ~1.08us/step of matmul, ACT ~1.07us/step of exp) stay
saturated end to end:

  * attention starts as soon as slice 0 of Q/K/V is projected (~8us) instead
    of after the full batch-0 projection (~35us); the remaining batch-0
    projection units run as in-loop fillers, DMA-paced.
  * softmax reciprocal on DVE (reciprocal_approx_fast, 1 inst/head) instead
    of the ACT Ln/Exp chain -- ACT runs a pure stream of 256 big exps.
  * next batch's projections are queued mid-batch and capped at one unit per
    step; when the filler queue runs dry (batch 3 has no next batch), tiny
    warm-keeper matmuls stop the HAM clock gate from re-throttling the PE
    (v2 lost ~40us to 148 cold matmuls + 12 mid-kernel re-throttles).

Device algorithm per core (all matmuls bf16, f32 PSUM):
  1. QT/KT = Wc^T x^T + b  -> SBUF [128=d', 8192=s] bf16 (h0 rows 0:64,
     h1 rows 64:128); V -> SBUF [token, d'] chunks with ones columns for the
     softmax row-sum rows (VE layout [128, chunk, 2, 132]).
  2. Per (batch, q-slice of 512): 16 k-chunks of 128: scores^T pair
     (row-tiled concurrent, h0 rows 0:64 / h1 rows 64:128) -> one ACT exp
     (N=1024) -> P^T; O^T accumulated per head via [V_h | ones] lhsT (row-sum
     row rides along). Normalize with DVE reciprocal + PE ones-broadcast ->
     OT bf16.
  3. out_partial = OT^T @ Wo per s-tile -> DRAM bf16.
"""

import os
from contextlib import ExitStack

import numpy as np
import ml_dtypes

import concourse.bass as bass
import concourse.mybir as mybir
import concourse.tile as tile
from concourse import bacc, library_config
from concourse.bass_utils import run_bass_kernel_spmd

B, S, D, H, HD = 4, 2048, 1024, 16, 64
BS = B * S                     # 8192 flattened tokens
NCORES = 8
HPC = H // NCORES              # 2 heads per core
DC = HPC * HD                  # 128-wide weight slice per core

F32 = mybir.dt.float32
BF16 = mybir.dt.bfloat16
EXP = mybir.ActivationFunctionType.Exp

_BUILT = None
LAST_EXEC_NS = None
LAST_RESULTS = None


def _build_program():
    nc = bacc.Bacc("TRN2", target_bir_lowering=False, debug=False,
                   num_devices=NCORES)

    qT_d = nc.dram_tensor("qT", [D, BS], BF16, kind="ExternalInput").ap()
    kT_d = nc.dram_tensor("kT", [D, BS], BF16, kind="ExternalInput").ap()
    vT_d = nc.dram_tensor("vT", [D, BS], BF16, kind="ExternalInput").ap()
    wq_d = nc.dram_tensor("wq", [D, DC], BF16, kind="ExternalInput").ap()
    wk_d = nc.dram_tensor("wk", [D, DC], BF16, kind="ExternalInput").ap()
    wv_d = nc.dram_tensor("wv", [D, DC], BF16, kind="ExternalInput").ap()
    wo_d = nc.dram_tensor("wo", [DC, D], BF16, kind="ExternalInput").ap()
    bq_d = nc.dram_tensor("bq", [DC, 1], F32, kind="ExternalInput").ap()
    bk_d = nc.dram_tensor("bk", [DC, 1], F32, kind="ExternalInput").ap()
    out_d = nc.dram_tensor("out", [BS, D], BF16, kind="ExternalOutput").ap()

    with tile.TileContext(nc) as tc, ExitStack() as ctx:
        const = ctx.enter_context(tc.tile_pool(name="const", bufs=1))
        persist = ctx.enter_context(tc.tile_pool(name="persist", bufs=1))
        stage = ctx.enter_context(tc.tile_pool(name="stage", bufs=3))
        ptpool = ctx.enter_context(tc.tile_pool(name="ptpool", bufs=6))
        npool = ctx.enter_context(tc.tile_pool(name="npool", bufs=2))
        ostage = ctx.enter_context(tc.tile_pool(name="ostage", bufs=3))
        # PSUM: psc 2 slots x 2 banks (scores double-buffer)
        #       acc 2 slots x 1 bank (proj accum / out-proj / bcast)
        #       pop 2 slots x 1 bank (per-head O^T accumulators)
        pscp = ctx.enter_context(tc.tile_pool(name="pscp", bufs=2, space="PSUM"))
        accp = ctx.enter_context(tc.tile_pool(name="accp", bufs=2, space="PSUM"))
        pop = ctx.enter_context(tc.tile_pool(name="pop", bufs=2, space="PSUM"))

        # ---- persistent SBUF state -------------------------------------
        QT = persist.tile([128, BS], BF16)          # [d', s]
        KT = persist.tile([128, BS], BF16)
        OT = persist.tile([128, BS], BF16)
        # V extended, per 128-token chunk (free layout [2, 132], abs width 264):
        #   abs cols 0:64    = V_h0          (h0 lhsT = abs 0:65, rsum row 64)
        #   abs col  64      = ones
        #   abs col  68      = ones          (h1 lhsT = abs 68:196, rsum row 0)
        #   abs cols 132:196 = V_h1          (-> h1 lhsT rows 64:128)
        VE = persist.tile([128, 64, 2, 132], BF16)

        # ---- constants --------------------------------------------------
        wq_sb = const.tile([128, 8, DC], BF16)
        wk_sb = const.tile([128, 8, DC], BF16)
        wv_sb = const.tile([128, 8, DC], BF16)
        wo_sb = const.tile([128, D], BF16)
        bq_sb = const.tile([128, 1], F32)
        bk_sb = const.tile([128, 1], F32)
        ones_sb = const.tile([128, 64], BF16)
        ones32_sb = const.tile([128, 64], F32)
        warm_sb = const.tile([128, 8], F32)
        nc.vector.memset(ones_sb[:], 1.0)
        nc.vector.memset(ones32_sb[:], 1.0)
        nc.vector.memset(warm_sb[:], 0.0)
        nc.sync.dma_start(wq_sb[:], wq_d.rearrange("(c p) d -> p c d", p=128))
        nc.sync.dma_start(wk_sb[:], wk_d.rearrange("(c p) d -> p c d", p=128))
        nc.sync.dma_start(wv_sb[:], wv_d.rearrange("(c p) d -> p c d", p=128))
        nc.sync.dma_start(wo_sb[:], wo_d)
        nc.sync.dma_start(bq_sb[:], bq_d)
        nc.sync.dma_start(bk_sb[:], bk_d)
        nc.vector.memset(VE[:], 0.0)
        nc.vector.memset(VE[:, :, 0, 64:65], 1.0)
        nc.vector.memset(VE[:, :, 0, 68:69], 1.0)
        # preload the exp table set during the DMA-bound lead-in
        nc.scalar.activation(warm_sb[:], warm_sb[:], EXP, scale=1.0)

        # ---- work-unit emitters -----------------------------------------
        def emit_qk_unit(which, ss):
            """Project one 512-token slice of Q or K (8 accum matmuls)."""
            srcT, w_sb, b_sb, dstT = (
                (qT_d, wq_sb, bq_sb, QT) if which == "q"
                else (kT_d, wk_sb, bk_sb, KT))
            xt = stage.tile([128, 8, 512], BF16, tag="xT")
            nc.sync.dma_start(
                xt[:],
                srcT.rearrange("(c p) s -> p c s", p=128)[
                    :, :, ss * 512:(ss + 1) * 512],
            )
            ps = accp.tile([128, 512], F32, tag="acc", name="psqk")
            for c in range(8):
                nc.tensor.matmul(ps[:], lhsT=w_sb[:, c], rhs=xt[:, c],
                                 start=(c == 0), stop=(c == 7))
            nc.vector.tensor_scalar_add(
                dstT[:, ss * 512:(ss + 1) * 512], ps[:], b_sb[:])

        def emit_v_load(ss):
            """DMA one 512-token slice of v; returns the staged tile."""
            vt = stage.tile([128, 8, 512], BF16, tag="xT")
            nc.sync.dma_start(
                vt[:],
                vT_d.rearrange("(c p) s -> p c s", p=128)[
                    :, :, ss * 512:(ss + 1) * 512],
            )
            return vt

        def emit_v_unit(vt, ss, st):
            """Project one 128-token chunk of V into the VE layout."""
            chunk = ss * 4 + st
            ps = accp.tile([128, 512], F32, tag="acc", name="psv")
            for c in range(8):
                nc.tensor.matmul(
                    ps[:, 0:DC],
                    lhsT=vt[:, c, st * 128:(st + 1) * 128],
                    rhs=wv_sb[:, c],
                    start=(c == 0), stop=(c == 7))
            nc.vector.tensor_copy(
                VE[:, chunk, :, 0:64],
                ps[:, 0:DC].rearrange("p (a x) -> p a x", a=2))

        def emit_outproj_unit(st_abs):
            """Out-projection for one 128-token tile (2 matmuls, bf16 out)."""
            s0 = st_abs * 128
            osb = ostage.tile([128, D], BF16, tag="osb")
            for ns in range(2):
                ps3 = accp.tile([128, 512], F32, tag="acc", name="ps3")
                nc.tensor.matmul(
                    ps3[:],
                    lhsT=OT[:, s0:s0 + 128],
                    rhs=wo_sb[:, ns * 512:(ns + 1) * 512],
                    start=True, stop=True)
                nc.vector.tensor_copy(osb[:, ns * 512:(ns + 1) * 512], ps3[:])
            nc.sync.dma_start(out_d[s0:s0 + 128, :], osb[:])

        # ---- attention step emitters ------------------------------------
        po_by_qs = {}
        posb_by_qs = {}
        pts = {}

        def emit_scores_exp(b, qs, chunk):
            q0 = b * S + qs * 512
            k0 = b * S + chunk * 128
            psc = pscp.tile([128, 2, 512], F32, tag="sc", name="psc")
            # row-tiled concurrent pair: h0 rows 0:64, h1 rows 64:128
            nc.tensor.matmul(
                psc[:, 0], lhsT=KT[0:64, k0:k0 + 128],
                rhs=QT[0:64, q0:q0 + 512], start=True, stop=True)
            nc.tensor.matmul(
                psc[:, 1], lhsT=KT[64:128, k0:k0 + 128],
                rhs=QT[64:128, q0:q0 + 512], start=True, stop=True)
            pt = ptpool.tile([128, 2, 512], BF16, tag="pt")
            nc.scalar.activation(
                pt.rearrange("p a x -> p (a x)"),
                psc.rearrange("p a x -> p (a x)"),
                EXP, scale=0.125)
            pts[(b, qs, chunk)] = pt

        def emit_pv(b, qs, chunk):
            if chunk == 0:
                po_by_qs[(b, qs)] = [
                    pop.tile([128, 512], F32, tag="po", name=f"po{h}")
                    for h in range(HPC)]
            po = po_by_qs[(b, qs)]
            pt = pts.pop((b, qs, chunk))
            ve_flat = VE[:, b * 16 + chunk, :, :].rearrange(
                "p a x -> p (a x)")
            first = chunk == 0
            last = chunk == 15
            # h0: rows 0:64 = O^T_h0, row 64 = rowsum_h0
            nc.tensor.matmul(
                po[0][0:65, :], lhsT=ve_flat[:, 0:65], rhs=pt[:, 0],
                start=first, stop=last)
            # h1: row 0 = rowsum_h1 (ones at abs 68), rows 64:128 = O^T_h1
            nc.tensor.matmul(
                po[1][:, :], lhsT=ve_flat[:, 68:196], rhs=pt[:, 1],
                start=first, stop=last)

        def emit_po_evac_and_recip(b, qs):
            # evacuate the finished accumulators to SBUF (frees both po
            # PSUM slots for the next q-slice's PV)
            po = po_by_qs.pop((b, qs))
            posb = [npool.tile([128, 512], F32, tag=f"posb{h}",
                               name=f"posb{h}")
                    for h in range(HPC)]
            nc.vector.tensor_copy(posb[0][0:65, :], po[0][0:65, :])
            nc.vector.tensor_copy(posb[1][:, :], po[1][:, :])
            posb_by_qs[(b, qs)] = posb

        def emit_norm_pe(b, qs):
            # col-tiled concurrent f32 ones-broadcast of the RAW row-sums,
            # one full-tile DVE reciprocal (base partition 0 -- the custom
            # DVE op mis-executes at base partition 64), then two DVE
            # multiplies write the normalized O^T
            q0 = b * S + qs * 512
            posb = posb_by_qs.pop((b, qs))
            bcp = accp.tile([128, 512], F32, tag="acc", name="bcp")
            nc.tensor.matmul(bcp[0:64, :], lhsT=ones32_sb[64:65, :],
                             rhs=posb[0][64:65, :], start=True, stop=True)
            nc.tensor.matmul(bcp[64:128, :], lhsT=ones32_sb[0:1, :],
                             rhs=posb[1][0:1, :], start=True, stop=True)
            rrs = npool.tile([128, 512], F32, tag="rrs")
            nc.vector.reciprocal_approx_fast(rrs[:], bcp[:])
            nc.vector.tensor_mul(
                OT[0:64, q0:q0 + 512], posb[0][0:64, :], rrs[0:64, :])
            nc.vector.tensor_mul(
                OT[64:128, q0:q0 + 512], posb[1][64:128, :],
                rrs[64:128, :])

        # ---- filler scheduling ------------------------------------------
        fillers = []

        def run_filler():
            """Pop and emit one filler unit; a unit may return a list of
            follow-up units which are queued to run next (in order).
            Returns True if a unit ran."""
            if not fillers:
                return False
            u = fillers.pop(0)
            r = u()
            if isinstance(r, list):
                fillers[0:0] = r
            return True

        def emit_warm_dummy():
            """Tiny matmul that keeps the HAM activity counter fed when the
            filler queue is dry (~25ns of PE, result never read)."""
            dps = accp.tile([128, 512], F32, tag="acc", name="warmmm")
            nc.tensor.matmul(dps[0:64, 0:64], lhsT=ones_sb[0:1, 0:64],
                             rhs=ones_sb[0:1, 0:64], start=True, stop=True)

        def proj_units_for_batch(b, k_first):
            """Projection units for batch b.

            k_first=True (lead-in residual): all K and V slices first --
            q-slice 0's scores/PV consume them within its first 16 steps --
            then the remaining Q slices.
            k_first=False (steady state): interleave [K, Q, V] per slice."""
            units = []

            def v_group(ss):
                vt = emit_v_load(ss)
                return [lambda st=st, vt=vt, ss=ss: emit_v_unit(vt, ss, st)
                        for st in range(4)]

            if k_first:
                for ss_local in range(1, 4):
                    ss = b * 4 + ss_local
                    units.append(lambda ss=ss: emit_qk_unit("k", ss))
                    units.append(lambda ss=ss: v_group(ss))
                for ss_local in range(1, 4):
                    ss = b * 4 + ss_local
                    units.append(lambda ss=ss: emit_qk_unit("q", ss))
            else:
                for ss_local in range(4):
                    ss = b * 4 + ss_local
                    units.append(lambda ss=ss: emit_qk_unit("k", ss))
                    units.append(lambda ss=ss: emit_qk_unit("q", ss))
                    units.append(lambda ss=ss: v_group(ss))
            return units

        # ---- lead-in: slice 0 of batch 0 only ---------------------------
        emit_qk_unit("k", 0)
        emit_qk_unit("q", 0)
        lead_v = emit_v_load(0)
        for st in range(4):
            emit_v_unit(lead_v, 0, st)
        fillers.extend(proj_units_for_batch(0, True))

        # ---- main pipeline: one continuous 256-step loop ----------------
        NSTEP = B * 64
        for gi in range(NSTEP + 6):
            if gi < NSTEP:
                b, r = divmod(gi, 64)
                qs, chunk = divmod(r, 16)
                emit_scores_exp(b, qs, chunk)
                if r == 8 and b + 1 < B:
                    # queue next batch's projections mid-batch
                    fillers.extend(proj_units_for_batch(b + 1, False))
            d = gi - 2
            if 0 <= d < NSTEP:
                db, dr = divmod(d, 64)
                dq, dc = divmod(dr, 16)
                emit_pv(db, dq, dc)
                if dc == 15:
                    emit_po_evac_and_recip(db, dq)
            e = gi - 5
            if 0 <= e < NSTEP and e % 16 == 15:
                eb, er = divmod(e, 64)
                eq = er // 16
                emit_norm_pe(eb, eq)
                base = (eb * S + eq * 512) // 128
                for k in range(4):
                    fillers.append(lambda st=base + k: emit_outproj_unit(st))
            # two fillers per step while the lead-in backlog drains, one after
            ran = run_filler()
            if gi < 16:
                run_filler()
            elif not ran and gi < NSTEP:
                emit_warm_dummy()

        # drain any unfinished fillers (last q-slice's out-projection)
        while fillers:
            run_filler()

    nc.compile()
    return nc


def _get_program():
    global _BUILT
    if _BUILT is None:
        _BUILT = _build_program()
    return _BUILT


def kernel(q, k, v, Wq, bq, Wk, bk, Wv, bv, Wo, bo, trace=None):
    global LAST_EXEC_NS, LAST_RESULTS
    if trace is None:
        trace = os.environ.get("KERNEL_TRACE", "0") == "1"
    bf16 = ml_dtypes.bfloat16

    q2 = np.asarray(q, np.float32).reshape(BS, D)
    k2 = np.asarray(k, np.float32).reshape(BS, D)
    v2 = np.asarray(v, np.float32).reshape(BS, D)
    qT = np.ascontiguousarray(q2.T).astype(bf16)
    kT = np.ascontiguousarray(k2.T).astype(bf16)
    vT = np.ascontiguousarray(v2.T).astype(bf16)

    Wq = np.asarray(Wq, np.float32)
    Wk = np.asarray(Wk, np.float32)
    Wv = np.asarray(Wv, np.float32)
    Wo = np.asarray(Wo, np.float32)
    bq = np.asarray(bq, np.float32)
    bk = np.asarray(bk, np.float32)
    bv = np.asarray(bv, np.float32)
    bo = np.asarray(bo, np.float32)

    in_maps = []
    for c in range(NCORES):
        sl = slice(c * DC, (c + 1) * DC)
        in_maps.append({
            "qT": qT, "kT": kT, "vT": vT,
            "wq": np.ascontiguousarray(Wq[:, sl]).astype(bf16),
            "wk": np.ascontiguousarray(Wk[:, sl]).astype(bf16),
            "wv": np.ascontiguousarray(Wv[:, sl]).astype(bf16),
            "wo": np.ascontiguousarray(Wo[sl, :]).astype(bf16),
            "bq": np.ascontiguousarray(bq[sl]).reshape(DC, 1),
            "bk": np.ascontiguousarray(bk[sl]).reshape(DC, 1),
        })

    nc = _get_program()
    res = run_bass_kernel_spmd(nc, in_maps, list(range(NCORES)), trace=trace)
    LAST_EXEC_NS = res.exec_time_ns
    LAST_RESULTS = res

    out = np.zeros((BS, D), np.float32)
    for c in range(NCORES):
        out += np.asarray(res.results[c]["out"], np.float32)
    out += bv.astype(np.float32) @ Wo + bo          # exact bias identities
    return out.reshape(B, S, D)
